# revision 30
# baseline (speedup 1.0000x reference)
"""Neural optimal transport kernel for 8 TRN2 NeuronCores.

Math (equivalent to the reference):
  hs = source @ W1[:256] + b1 ; ht = target @ W1[256:]
  C[i,j]  = relu(hs[i] + ht[j]) . W2 + b2
  K       = exp(-C)
  u       = (1/n) / (K @ b + eps)          b = ones/n  (constant every iter!)
  v       = (1/n) / (K^T @ u + eps)
  plan    = u[:,None] * K * v[None,:]
The reference's 100-iteration Sinkhorn loop is degenerate: the loop body
uses the constant marginal b (not v), so u and v are identical every
iteration; one evaluation gives the exact fixed point.

Sharding: rows of C/K (the i axis) are split 128-per-core across 8 cores.
K^T@u partial sums are combined with a single 4KB AllReduce.
"""

import numpy as np

N = 1024
EMBED = 256
HID = 128
EPS = 1e-8
NCORES = 8
ROWS = N // NCORES  # 128 rows of C per core
WAVES = ROWS // 4  # 4 rows per psum wave (partitions 0/32/64/96)

# Producer engine pattern, indexed by i % len: S=ScalarE, G=GpSimd, V=VectorE
PRODUCERS = "SGVVV"
DEBUG_TAPS = False
WARMUP_CC = True

_CACHE = {}


def _build():
    from contextlib import ExitStack

    import concourse.bass as bass
    import concourse.tile as tile
    from concourse import bacc, mybir

    f32 = mybir.dt.float32
    bf16 = mybir.dt.bfloat16

    nc = bacc.Bacc("TRN2", num_devices=NCORES, debug=False)

    srcT = nc.dram_tensor("srcT_blk", [EMBED, ROWS], bf16, kind="ExternalInput").ap()
    tgtT = nc.dram_tensor("tgtT", [EMBED, N], bf16, kind="ExternalInput").ap()
    w1a = nc.dram_tensor("w1a", [EMBED, HID], bf16, kind="ExternalInput").ap()
    w1b = nc.dram_tensor("w1b", [EMBED, HID], bf16, kind="ExternalInput").ap()
    b1d = nc.dram_tensor("b1", [HID], f32, kind="ExternalInput").ap()
    w2d = nc.dram_tensor("w2", [HID], f32, kind="ExternalInput").ap()
    b2d = nc.dram_tensor("b2", [1], f32, kind="ExternalInput").ap()
    C_out = nc.dram_tensor("C_blk", [ROWS, N], f32, kind="ExternalOutput").ap()
    P_out = nc.dram_tensor("plan_blk", [ROWS, N], f32, kind="ExternalOutput").ap()
    dbg_out = None
    if DEBUG_TAPS:
        dbg_out = nc.dram_tensor("dbg", [3, 128, 8], f32, kind="ExternalOutput").ap()

    from concourse.masks import make_identity

    with tile.TileContext(nc) as tc, ExitStack() as ctx:
        singles = ctx.enter_context(tc.tile_pool(name="singles", bufs=1))
        mpool = ctx.enter_context(tc.tile_pool(name="mpool", bufs=6))
        ps1 = ctx.enter_context(tc.tile_pool(name="ps1", bufs=1, space="PSUM"))
        psct = ctx.enter_context(tc.tile_pool(name="psct", bufs=1, space="PSUM"))
        pstp = ctx.enter_context(tc.tile_pool(name="pstp", bufs=1, space="PSUM"))
        psv = ctx.enter_context(tc.tile_pool(name="psv", bufs=1, space="PSUM"))
        dram = ctx.enter_context(tc.tile_pool(name="dram", bufs=1, space="DRAM"))

        # warm-up AllGather: no data deps, so it schedules early and warms
        # the collective firmware path before the real gather at the tail
        if WARMUP_CC:
            wu_in = dram.tile([128, 1], f32, tag="wu_in")
            wu_out = dram.tile(
                [NCORES * 128, 1], f32, tag="wu_out", addr_space="Shared"
            )
            wu_sb = singles.tile([128, 1], f32, tag="wu_sb")
            nc.vector.memset(wu_sb, 0.0)
            nc.gpsimd.dma_start(out=wu_in[:], in_=wu_sb)
            nc.gpsimd.collective_compute(
                "AllGather",
                mybir.AluOpType.bypass,
                replica_groups=[list(range(NCORES))],
                ins=[wu_in[:].opt()],
                outs=[wu_out[:].opt()],
            )

        # ---- load inputs ------------------------------------------------
        # tgtT is the big input (512KB): split into 4 DMAs so transfers
        # spread across queues and stage-1 matmuls start sooner
        tgt_sb = singles.tile([128, 2, N], bf16, tag="tgt_sb")
        tgtT_r = tgtT.rearrange("(a p) n -> p a n", p=128)
        for k in range(2):
            for h in range(2):
                sl = slice(512 * h, 512 * h + 512)
                nc.sync.dma_start(out=tgt_sb[:, k, sl], in_=tgtT_r[:, k, sl])
        src_sb = singles.tile([128, 2, ROWS], bf16, tag="src_sb")
        nc.sync.dma_start(out=src_sb, in_=srcT.rearrange("(a p) n -> p a n", p=128))
        w1a_sb = singles.tile([128, 2, HID], bf16, tag="w1a_sb")
        nc.sync.dma_start(out=w1a_sb, in_=w1a.rearrange("(a p) n -> p a n", p=128))
        w1b_sb = singles.tile([128, 2, HID], bf16, tag="w1b_sb")
        nc.sync.dma_start(out=w1b_sb, in_=w1b.rearrange("(a p) n -> p a n", p=128))
        b1_sb = singles.tile([128, 1], f32, tag="b1_sb")
        nc.sync.dma_start(out=b1_sb, in_=b1d.rearrange("(p a) -> p a", a=1))
        w2f = singles.tile([128, 1], f32, tag="w2f")
        nc.sync.dma_start(out=w2f, in_=w2d.rearrange("(p a) -> p a", a=1))
        b2_sb = singles.tile([128, 1], f32, tag="b2_sb")
        nc.sync.dma_start(out=b2_sb, in_=b2d.partition_broadcast(128))

        w2b = singles.tile([128, 1], bf16, tag="w2b")
        nc.vector.tensor_copy(w2b, w2f)
        negb2 = singles.tile([128, 1], f32, tag="negb2")
        nc.vector.tensor_scalar_mul(negb2, b2_sb, -1.0)

        # ---- stage 1: hsT [h, i_blk], htT [h, j] via PE ------------------
        ht_ps = ps1.tile([128, N], f32, tag="ht_ps")
        for h in range(2):
            sl = slice(512 * h, 512 * h + 512)
            for k in range(2):
                nc.tensor.matmul(
                    out=ht_ps[:, sl],
                    lhsT=w1b_sb[:, k, :],
                    rhs=tgt_sb[:, k, sl],
                    start=(k == 0),
                    stop=(k == 1),
                )
        hs_ps = ps1.tile([128, ROWS], f32, tag="hs_ps")
        for k in range(2):
            nc.tensor.matmul(
                out=hs_ps,
                lhsT=w1a_sb[:, k, :],
                rhs=src_sb[:, k, :],
                start=(k == 0),
                stop=(k == 1),
            )

        htT_f32 = singles.tile([128, N], f32, tag="htT_f32")
        nc.scalar.copy(htT_f32, ht_ps)
        htT_b = singles.tile([128, N], bf16, tag="htT_b")
        nc.vector.tensor_copy(htT_b, ht_ps)
        # hs' = hs + b1 (per-partition over h)
        hs_f32 = singles.tile([128, ROWS], f32, tag="hs_f32")
        nc.vector.tensor_scalar_add(hs_f32, hs_ps, b1_sb)

        # ---- stage 2: C^T columns via per-(i, jc) stationary matmuls -----
        # stationary = relu-tile chunk [128_h, 128_j], moving = W2 [128_h, 1]
        # -> out [128_j, 1] = column i of C^T chunk jc, dense in partitions.
        add = mybir.AluOpType.add
        mx = mybir.AluOpType.max
        mult = mybir.AluOpType.mult
        relu = mybir.ActivationFunctionType.Relu

        ident = singles.tile([128, 128], f32, tag="ident")
        make_identity(nc, ident)

        CT_ps = psct.tile([128, N], f32, tag="CT_ps")  # [j%128, jc*128 + i]
        for i in range(ROWS):
            m_t = mpool.tile([128, N], bf16, tag="m_t")
            eng = PRODUCERS[i % len(PRODUCERS)]
            if eng == "S":
                nc.scalar.activation(
                    m_t, htT_f32, relu, bias=hs_f32[:, i : i + 1], scale=1.0
                )
            elif eng == "G":
                nc.gpsimd.tensor_scalar(
                    m_t, htT_b, hs_f32[:, i : i + 1], 0.0, add, mx
                )
            else:
                nc.vector.tensor_scalar(
                    m_t, htT_b, hs_f32[:, i : i + 1], 0.0, add, mx
                )
            for jc in range(8):
                nc.tensor.matmul(
                    out=CT_ps[:, 128 * jc + i : 128 * jc + i + 1],
                    lhsT=m_t[:, 128 * jc : 128 * jc + 128],
                    rhs=w2b,
                    start=True,
                    stop=True,
                )

        # evacuate C^T, transpose back to row-major via PE
        CT_sb = singles.tile([128, N], f32, tag="CT_sb")
        nc.vector.tensor_copy(CT_sb, CT_ps)
        TP_ps = pstp.tile([128, N], f32, tag="TP_ps")
        for jc in range(8):
            nc.tensor.transpose(
                TP_ps[:, 128 * jc : 128 * jc + 128],
                CT_sb[:, 128 * jc : 128 * jc + 128],
                ident,
            )
        Kpre = singles.tile([128, N], f32, tag="Kpre")  # C - b2, row-major
        nc.scalar.copy(Kpre, TP_ps)

        # C output (without b2; b2 is re-added host-side iff nonzero)
        for h in range(2):
            sl = slice(512 * h, 512 * h + 512)
            nc.sync.dma_start(out=C_out[:, sl], in_=Kpre[:, sl])

        # ---- stage 3: K = exp(-(C+b2)) fused with row-sum ----------------
        K_sb = singles.tile([128, N], f32, tag="K_sb")
        rs = singles.tile([128, 1], f32, tag="rs")
        nc.scalar.activation(
            K_sb,
            Kpre,
            mybir.ActivationFunctionType.Exp,
            bias=negb2,
            scale=-1.0,
            accum_out=rs,
        )
        # u = (1/n) / (rowsum/n + eps)
        t1 = singles.tile([128, 1], f32, tag="t1")
        nc.vector.tensor_scalar(t1, rs, 1.0 / N, EPS, mult, add)
        rcp1 = singles.tile([128, 1], f32, tag="rcp1")
        nc.vector.reciprocal(rcp1, t1)
        u_sb = singles.tile([128, 1], f32, tag="u_sb")
        nc.vector.tensor_scalar_mul(u_sb, rcp1, 1.0 / N)

        # ---- stage 4: v_denom partial = K^T @ u, AllGather + local sum ---
        cc_in = dram.tile([128, 8], f32, tag="cc_in")
        ag_out = dram.tile([NCORES * 128, 8], f32, tag="ag_out", addr_space="Shared")
        # bf16 matvec: the f32 M=1 matmul path yields wrong results on HW
        u_b = singles.tile([128, 1], bf16, tag="u_b")
        nc.vector.tensor_copy(u_b, u_sb)
        K_b = singles.tile([128, N], bf16, tag="K_b")
        nc.vector.tensor_copy(K_b, K_sb)
        v_row = singles.tile([1, N], f32, tag="v_row")
        for h in range(2):
            vp = psv.tile([1, 512], f32, tag="vp")
            nc.tensor.matmul(
                out=vp, lhsT=u_b, rhs=K_b[:, 512 * h : 512 * h + 512],
                start=True, stop=True,
            )
            nc.vector.tensor_copy(v_row[:, 512 * h : 512 * h + 512], vp)
        # v_row [1, 1024] SBUF -> cc_in [128, 8] DRAM: DMA copies in linear AP
        # order, so the reshape happens via the DRAM-side view. (Folding SBUF
        # free offsets into partitions via rearrange reads garbage on HW.)
        nc.gpsimd.dma_start(out=cc_in[:], in_=v_row)
        nc.gpsimd.collective_compute(
            "AllGather",
            mybir.AluOpType.bypass,
            replica_groups=[list(range(NCORES))],
            ins=[cc_in[:].opt()],
            outs=[ag_out[:].opt()],
        )
        # local sum of the 8 gathered partials: [128, 8(j-chunk), 8(core)]
        vg = singles.tile([128, 8, NCORES], f32, tag="vg")
        nc.gpsimd.dma_start(
            out=vg, in_=ag_out[:].rearrange("(a p) c -> p c a", p=128)
        )
        vd = singles.tile([128, 8], f32, tag="vd")
        nc.vector.tensor_reduce(
            vd, vg, axis=mybir.AxisListType.X, op=mybir.AluOpType.add
        )
        # v = (1/n) / (v_denom + eps)   computed at [128, 8]
        t2 = singles.tile([128, 8], f32, tag="t2")
        nc.vector.tensor_scalar_add(t2, vd, EPS)
        rcp2 = singles.tile([128, 8], f32, tag="rcp2")
        nc.vector.reciprocal(rcp2, t2)
        v_sb = singles.tile([128, 8], f32, tag="v_sb")
        nc.vector.tensor_scalar_mul(v_sb, rcp2, 1.0 / N)
        # roundtrip through DRAM to reshape [128, 8] -> one [1, 1024] row,
        # then PE-broadcast (ones outer product) to all 128 partitions
        v_dram = dram.tile([128, 8], f32, tag="v_dram")
        nc.gpsimd.dma_start(out=v_dram[:], in_=v_sb)
        v_row2 = singles.tile([1, N], f32, tag="v_row2")
        nc.gpsimd.dma_start(out=v_row2, in_=v_dram[:].rearrange("p c -> (p c)"))
        v_row_b = singles.tile([1, N], bf16, tag="v_row_b")
        nc.vector.tensor_copy(v_row_b, v_row2)
        ones_b = singles.tile([1, 128], bf16, tag="ones_b")
        nc.vector.memset(ones_b, 1.0)
        # reuses the C^T psum slot (same tag): CT_ps is long dead by now
        V_b = psct.tile([128, N], f32, tag="CT_ps", name="V_b")
        for h in range(2):
            sl = slice(512 * h, 512 * h + 512)
            nc.tensor.matmul(
                out=V_b[:, sl], lhsT=ones_b, rhs=v_row_b[:, sl],
                start=True, stop=True,
            )

        if DEBUG_TAPS:
            # debug taps: local v partial, summed denom, u (repeated x8)
            nc.sync.dma_start(out=dbg_out[0], in_=v_row)
            nc.sync.dma_start(out=dbg_out[1], in_=vd)
            u_rep = singles.tile([128, 8], f32, tag="u_rep")
            nc.vector.tensor_scalar(u_rep, vd, 0.0, u_sb, mult, add)
            nc.sync.dma_start(out=dbg_out[2], in_=u_rep)

        # ---- stage 5: plan = (u * K) * v ---------------------------------
        # u*K has no dependency on the collective result, so the scheduler
        # can run it inside the AllGather window.
        plan1 = singles.tile([128, N], f32, tag="plan1")
        nc.vector.tensor_scalar_mul(plan1, K_sb, u_sb)
        plan2 = singles.tile([128, N], f32, tag="plan2")
        nc.vector.tensor_mul(plan2, plan1, V_b)
        for h in range(2):
            sl = slice(512 * h, 512 * h + 512)
            nc.sync.dma_start(out=P_out[:, sl], in_=plan2[:, sl])

    nc.compile()
    return nc


def _get_nc():
    if "nc" not in _CACHE:
        _CACHE["nc"] = _build()
    return _CACHE["nc"]


def make_in_maps(source, target, W1, b1, W2, b2):
    import ml_dtypes

    f = np.float32
    bf = ml_dtypes.bfloat16
    tgtT = np.ascontiguousarray(np.asarray(target, f).T).astype(bf)
    w1a = np.ascontiguousarray(np.asarray(W1, f)[:EMBED]).astype(bf)
    w1b = np.ascontiguousarray(np.asarray(W1, f)[EMBED:]).astype(bf)
    b1v = np.ascontiguousarray(np.asarray(b1, f))
    w2v = np.ascontiguousarray(np.asarray(W2, f)[:, 0])
    b2v = np.ascontiguousarray(np.asarray(b2, f))
    src = np.asarray(source, f)
    maps = []
    for c in range(NCORES):
        maps.append(
            {
                "srcT_blk": np.ascontiguousarray(
                    src[c * ROWS : (c + 1) * ROWS].T
                ).astype(bf),
                "tgtT": tgtT,
                "w1a": w1a,
                "w1b": w1b,
                "b1": b1v,
                "w2": w2v,
                "b2": b2v,
            }
        )
    return maps


def _patch_ldw_opt():
    """Rewrite --enable-ldw-opt=false -> true in the walrus invocation
    (enables fast weight load; gated on KERNEL_LDW_OPT=1 for A/B)."""
    import os

    if os.environ.get("KERNEL_LDW_OPT") != "1" or _CACHE.get("ldw_patched"):
        return
    from concourse import bass_utils

    orig = bass_utils.run_command

    def patched(argv, **kwargs):
        argv = [
            "--enable-ldw-opt=true" if a == "--enable-ldw-opt=false" else a
            for a in argv
        ]
        return orig(argv, **kwargs)

    bass_utils.run_command = patched
    _CACHE["ldw_patched"] = True


def run(inputs, trace=False):
    """Run the SPMD kernel; returns ((plan, C), exec_time_ns_or_None)."""
    from concourse import bass_utils

    _patch_ldw_opt()

    nc = _get_nc()
    in_maps = make_in_maps(**inputs)
    res = bass_utils.run_bass_kernel_spmd(
        nc, in_maps, list(range(NCORES)), trace=trace
    )
    _CACHE["last_res"] = res
    plan = np.concatenate([res.results[c]["plan_blk"] for c in range(NCORES)], axis=0)
    C = np.concatenate([res.results[c]["C_blk"] for c in range(NCORES)], axis=0)
    b2v = float(np.asarray(inputs["b2"], np.float64)[0])
    if b2v != 0.0:
        C = C + np.float32(b2v)
    return (plan, C), res.exec_time_ns


def kernel(source, target, W1, b1, W2, b2):
    (plan, C), _ = run(
        dict(source=source, target=target, W1=W1, b1=b1, W2=W2, b2=b2)
    )
    return plan, C


# revision 31
# speedup vs baseline: 3.9881x; 3.9881x over previous
"""Neural optimal transport kernel for 8 TRN2 NeuronCores.

Math (equivalent to the reference):
  hs = source @ W1[:256] + b1 ; ht = target @ W1[256:]
  C[i,j]  = relu(hs[i] + ht[j]) . W2 + b2
  K       = exp(-C)
  u       = (1/n) / (K @ b + eps)          b = ones/n  (constant every iter!)
  v       = (1/n) / (K^T @ u + eps)
  plan    = u[:,None] * K * v[None,:]
The reference's 100-iteration Sinkhorn loop is degenerate: the loop body
uses the constant marginal b (not v), so u and v are identical every
iteration; one evaluation gives the exact fixed point.

Sharding: rows of C/K (the i axis) are split 128-per-core across 8 cores.
K^T@u partial sums are combined with a single 4KB AllReduce.
"""

import numpy as np

N = 1024
EMBED = 256
HID = 128
EPS = 1e-8
NCORES = 8
ROWS = N // NCORES  # 128 rows of C per core
WAVES = ROWS // 4  # 4 rows per psum wave (partitions 0/32/64/96)

# Producer engine pattern, indexed by i % len: S=ScalarE, G=GpSimd, V=VectorE
# (GpSimd measured ~28us per [128,1024] tensor_scalar on HW — never use it)
PRODUCERS = "SVVVSVVVSV"
DEBUG_TAPS = False
WARMUP_CC = True

_CACHE = {}


def _build():
    from contextlib import ExitStack

    import concourse.bass as bass
    import concourse.tile as tile
    from concourse import bacc, mybir

    f32 = mybir.dt.float32
    bf16 = mybir.dt.bfloat16

    nc = bacc.Bacc("TRN2", num_devices=NCORES, debug=False)

    srcT = nc.dram_tensor("srcT_blk", [EMBED, ROWS], bf16, kind="ExternalInput").ap()
    tgtT = nc.dram_tensor("tgtT", [EMBED, N], bf16, kind="ExternalInput").ap()
    w1a = nc.dram_tensor("w1a", [EMBED, HID], bf16, kind="ExternalInput").ap()
    w1b = nc.dram_tensor("w1b", [EMBED, HID], bf16, kind="ExternalInput").ap()
    b1d = nc.dram_tensor("b1", [HID], f32, kind="ExternalInput").ap()
    w2d = nc.dram_tensor("w2", [HID], f32, kind="ExternalInput").ap()
    b2d = nc.dram_tensor("b2", [1], f32, kind="ExternalInput").ap()
    C_out = nc.dram_tensor("C_blk", [ROWS, N], f32, kind="ExternalOutput").ap()
    P_out = nc.dram_tensor("plan_blk", [ROWS, N], f32, kind="ExternalOutput").ap()
    dbg_out = None
    if DEBUG_TAPS:
        dbg_out = nc.dram_tensor("dbg", [3, 128, 8], f32, kind="ExternalOutput").ap()

    from concourse.masks import make_identity

    with tile.TileContext(nc) as tc, ExitStack() as ctx:
        singles = ctx.enter_context(tc.tile_pool(name="singles", bufs=1))
        mpool = ctx.enter_context(tc.tile_pool(name="mpool", bufs=6))
        ps1 = ctx.enter_context(tc.tile_pool(name="ps1", bufs=1, space="PSUM"))
        psct = ctx.enter_context(tc.tile_pool(name="psct", bufs=1, space="PSUM"))
        pstp = ctx.enter_context(tc.tile_pool(name="pstp", bufs=1, space="PSUM"))
        psv = ctx.enter_context(tc.tile_pool(name="psv", bufs=1, space="PSUM"))
        dram = ctx.enter_context(tc.tile_pool(name="dram", bufs=1, space="DRAM"))

        # warm-up AllGather: no data deps, so it schedules early and warms
        # the collective firmware path before the real gather at the tail
        if WARMUP_CC:
            wu_in = dram.tile([128, 1], f32, tag="wu_in")
            wu_out = dram.tile(
                [NCORES * 128, 1], f32, tag="wu_out", addr_space="Shared"
            )
            wu_sb = singles.tile([128, 1], f32, tag="wu_sb")
            nc.vector.memset(wu_sb, 0.0)
            nc.gpsimd.dma_start(out=wu_in[:], in_=wu_sb)
            nc.gpsimd.collective_compute(
                "AllGather",
                mybir.AluOpType.bypass,
                replica_groups=[list(range(NCORES))],
                ins=[wu_in[:].opt()],
                outs=[wu_out[:].opt()],
            )

        # ---- load inputs ------------------------------------------------
        # tgtT is the big input (512KB): split into 4 DMAs so transfers
        # spread across queues and stage-1 matmuls start sooner
        tgt_sb = singles.tile([128, 2, N], bf16, tag="tgt_sb")
        tgtT_r = tgtT.rearrange("(a p) n -> p a n", p=128)
        for k in range(2):
            for h in range(2):
                sl = slice(512 * h, 512 * h + 512)
                nc.sync.dma_start(out=tgt_sb[:, k, sl], in_=tgtT_r[:, k, sl])
        src_sb = singles.tile([128, 2, ROWS], bf16, tag="src_sb")
        nc.sync.dma_start(out=src_sb, in_=srcT.rearrange("(a p) n -> p a n", p=128))
        w1a_sb = singles.tile([128, 2, HID], bf16, tag="w1a_sb")
        nc.sync.dma_start(out=w1a_sb, in_=w1a.rearrange("(a p) n -> p a n", p=128))
        w1b_sb = singles.tile([128, 2, HID], bf16, tag="w1b_sb")
        nc.sync.dma_start(out=w1b_sb, in_=w1b.rearrange("(a p) n -> p a n", p=128))
        b1_sb = singles.tile([128, 1], f32, tag="b1_sb")
        nc.sync.dma_start(out=b1_sb, in_=b1d.rearrange("(p a) -> p a", a=1))
        w2f = singles.tile([128, 1], f32, tag="w2f")
        nc.sync.dma_start(out=w2f, in_=w2d.rearrange("(p a) -> p a", a=1))
        b2_sb = singles.tile([128, 1], f32, tag="b2_sb")
        nc.sync.dma_start(out=b2_sb, in_=b2d.partition_broadcast(128))

        w2b = singles.tile([128, 1], bf16, tag="w2b")
        nc.vector.tensor_copy(w2b, w2f)
        negb2 = singles.tile([128, 1], f32, tag="negb2")
        nc.vector.tensor_scalar_mul(negb2, b2_sb, -1.0)

        # ---- stage 1: hsT [h, i_blk], htT [h, j] via PE ------------------
        ht_ps = ps1.tile([128, N], f32, tag="ht_ps")
        for h in range(2):
            sl = slice(512 * h, 512 * h + 512)
            for k in range(2):
                nc.tensor.matmul(
                    out=ht_ps[:, sl],
                    lhsT=w1b_sb[:, k, :],
                    rhs=tgt_sb[:, k, sl],
                    start=(k == 0),
                    stop=(k == 1),
                )
        hs_ps = ps1.tile([128, ROWS], f32, tag="hs_ps")
        for k in range(2):
            nc.tensor.matmul(
                out=hs_ps,
                lhsT=w1a_sb[:, k, :],
                rhs=src_sb[:, k, :],
                start=(k == 0),
                stop=(k == 1),
            )

        htT_f32 = singles.tile([128, N], f32, tag="htT_f32")
        nc.scalar.copy(htT_f32, ht_ps)
        htT_b = singles.tile([128, N], bf16, tag="htT_b")
        nc.vector.tensor_copy(htT_b, ht_ps)
        # hs' = hs + b1 (per-partition over h)
        hs_f32 = singles.tile([128, ROWS], f32, tag="hs_f32")
        nc.vector.tensor_scalar_add(hs_f32, hs_ps, b1_sb)

        # ---- stage 2: C^T columns via per-(i, jc) stationary matmuls -----
        # stationary = relu-tile chunk [128_h, 128_j], moving = W2 [128_h, 1]
        # -> out [128_j, 1] = column i of C^T chunk jc, dense in partitions.
        add = mybir.AluOpType.add
        mx = mybir.AluOpType.max
        mult = mybir.AluOpType.mult
        relu = mybir.ActivationFunctionType.Relu

        ident = singles.tile([128, 128], f32, tag="ident")
        make_identity(nc, ident)

        CT_ps = psct.tile([128, N], f32, tag="CT_ps")  # [j%128, jc*128 + i]
        for i in range(ROWS):
            m_t = mpool.tile([128, N], bf16, tag="m_t")
            eng = PRODUCERS[i % len(PRODUCERS)]
            if eng == "S":
                nc.scalar.activation(
                    m_t, htT_f32, relu, bias=hs_f32[:, i : i + 1], scale=1.0
                )
            elif eng == "G":
                nc.gpsimd.tensor_scalar(
                    m_t, htT_b, hs_f32[:, i : i + 1], 0.0, add, mx
                )
            else:
                nc.vector.tensor_scalar(
                    m_t, htT_b, hs_f32[:, i : i + 1], 0.0, add, mx
                )
            for jc in range(8):
                nc.tensor.matmul(
                    out=CT_ps[:, 128 * jc + i : 128 * jc + i + 1],
                    lhsT=m_t[:, 128 * jc : 128 * jc + 128],
                    rhs=w2b,
                    start=True,
                    stop=True,
                )

        # evacuate C^T, transpose back to row-major via PE
        CT_sb = singles.tile([128, N], f32, tag="CT_sb")
        nc.vector.tensor_copy(CT_sb, CT_ps)
        TP_ps = pstp.tile([128, N], f32, tag="TP_ps")
        for jc in range(8):
            nc.tensor.transpose(
                TP_ps[:, 128 * jc : 128 * jc + 128],
                CT_sb[:, 128 * jc : 128 * jc + 128],
                ident,
            )
        Kpre = singles.tile([128, N], f32, tag="Kpre")  # C - b2, row-major
        nc.scalar.copy(Kpre, TP_ps)

        # C output (without b2; b2 is re-added host-side iff nonzero)
        for h in range(2):
            sl = slice(512 * h, 512 * h + 512)
            nc.sync.dma_start(out=C_out[:, sl], in_=Kpre[:, sl])

        # ---- stage 3: K = exp(-(C+b2)) fused with row-sum ----------------
        K_sb = singles.tile([128, N], f32, tag="K_sb")
        rs = singles.tile([128, 1], f32, tag="rs")
        nc.scalar.activation(
            K_sb,
            Kpre,
            mybir.ActivationFunctionType.Exp,
            bias=negb2,
            scale=-1.0,
            accum_out=rs,
        )
        # u = (1/n) / (rowsum/n + eps)
        t1 = singles.tile([128, 1], f32, tag="t1")
        nc.vector.tensor_scalar(t1, rs, 1.0 / N, EPS, mult, add)
        rcp1 = singles.tile([128, 1], f32, tag="rcp1")
        nc.vector.reciprocal(rcp1, t1)
        u_sb = singles.tile([128, 1], f32, tag="u_sb")
        nc.vector.tensor_scalar_mul(u_sb, rcp1, 1.0 / N)

        # ---- stage 4: v_denom partial = K^T @ u, AllGather + local sum ---
        cc_in = dram.tile([128, 8], f32, tag="cc_in")
        ag_out = dram.tile([NCORES * 128, 8], f32, tag="ag_out", addr_space="Shared")
        # bf16 matvec: the f32 M=1 matmul path yields wrong results on HW
        u_b = singles.tile([128, 1], bf16, tag="u_b")
        nc.vector.tensor_copy(u_b, u_sb)
        K_b = singles.tile([128, N], bf16, tag="K_b")
        nc.vector.tensor_copy(K_b, K_sb)
        v_row = singles.tile([1, N], f32, tag="v_row")
        for h in range(2):
            vp = psv.tile([1, 512], f32, tag="vp")
            nc.tensor.matmul(
                out=vp, lhsT=u_b, rhs=K_b[:, 512 * h : 512 * h + 512],
                start=True, stop=True,
            )
            nc.vector.tensor_copy(v_row[:, 512 * h : 512 * h + 512], vp)
        # v_row [1, 1024] SBUF -> cc_in [128, 8] DRAM: DMA copies in linear AP
        # order, so the reshape happens via the DRAM-side view. (Folding SBUF
        # free offsets into partitions via rearrange reads garbage on HW.)
        nc.gpsimd.dma_start(out=cc_in[:], in_=v_row)
        nc.gpsimd.collective_compute(
            "AllGather",
            mybir.AluOpType.bypass,
            replica_groups=[list(range(NCORES))],
            ins=[cc_in[:].opt()],
            outs=[ag_out[:].opt()],
        )
        # local sum of the 8 gathered partials: [128, 8(j-chunk), 8(core)]
        vg = singles.tile([128, 8, NCORES], f32, tag="vg")
        nc.gpsimd.dma_start(
            out=vg, in_=ag_out[:].rearrange("(a p) c -> p c a", p=128)
        )
        vd = singles.tile([128, 8], f32, tag="vd")
        nc.vector.tensor_reduce(
            vd, vg, axis=mybir.AxisListType.X, op=mybir.AluOpType.add
        )
        # v = (1/n) / (v_denom + eps)   computed at [128, 8]
        t2 = singles.tile([128, 8], f32, tag="t2")
        nc.vector.tensor_scalar_add(t2, vd, EPS)
        rcp2 = singles.tile([128, 8], f32, tag="rcp2")
        nc.vector.reciprocal(rcp2, t2)
        v_sb = singles.tile([128, 8], f32, tag="v_sb")
        nc.vector.tensor_scalar_mul(v_sb, rcp2, 1.0 / N)
        # roundtrip through DRAM to reshape [128, 8] -> one [1, 1024] row,
        # then PE-broadcast (ones outer product) to all 128 partitions
        v_dram = dram.tile([128, 8], f32, tag="v_dram")
        nc.gpsimd.dma_start(out=v_dram[:], in_=v_sb)
        v_row2 = singles.tile([1, N], f32, tag="v_row2")
        nc.gpsimd.dma_start(out=v_row2, in_=v_dram[:].rearrange("p c -> (p c)"))
        v_row_b = singles.tile([1, N], bf16, tag="v_row_b")
        nc.vector.tensor_copy(v_row_b, v_row2)
        ones_b = singles.tile([1, 128], bf16, tag="ones_b")
        nc.vector.memset(ones_b, 1.0)
        # reuses the C^T psum slot (same tag): CT_ps is long dead by now
        V_b = psct.tile([128, N], f32, tag="CT_ps", name="V_b")
        for h in range(2):
            sl = slice(512 * h, 512 * h + 512)
            nc.tensor.matmul(
                out=V_b[:, sl], lhsT=ones_b, rhs=v_row_b[:, sl],
                start=True, stop=True,
            )

        if DEBUG_TAPS:
            # debug taps: local v partial, summed denom, u (repeated x8)
            nc.sync.dma_start(out=dbg_out[0], in_=v_row)
            nc.sync.dma_start(out=dbg_out[1], in_=vd)
            u_rep = singles.tile([128, 8], f32, tag="u_rep")
            nc.vector.tensor_scalar(u_rep, vd, 0.0, u_sb, mult, add)
            nc.sync.dma_start(out=dbg_out[2], in_=u_rep)

        # ---- stage 5: plan = (u * K) * v ---------------------------------
        # u*K has no dependency on the collective result, so the scheduler
        # can run it inside the AllGather window.
        plan1 = singles.tile([128, N], f32, tag="plan1")
        nc.vector.tensor_scalar_mul(plan1, K_sb, u_sb)
        plan2 = singles.tile([128, N], f32, tag="plan2")
        nc.vector.tensor_mul(plan2, plan1, V_b)
        for h in range(2):
            sl = slice(512 * h, 512 * h + 512)
            nc.sync.dma_start(out=P_out[:, sl], in_=plan2[:, sl])

    nc.compile()
    return nc


def _get_nc():
    if "nc" not in _CACHE:
        _CACHE["nc"] = _build()
    return _CACHE["nc"]


def make_in_maps(source, target, W1, b1, W2, b2):
    import ml_dtypes

    f = np.float32
    bf = ml_dtypes.bfloat16
    tgtT = np.ascontiguousarray(np.asarray(target, f).T).astype(bf)
    w1a = np.ascontiguousarray(np.asarray(W1, f)[:EMBED]).astype(bf)
    w1b = np.ascontiguousarray(np.asarray(W1, f)[EMBED:]).astype(bf)
    b1v = np.ascontiguousarray(np.asarray(b1, f))
    w2v = np.ascontiguousarray(np.asarray(W2, f)[:, 0])
    b2v = np.ascontiguousarray(np.asarray(b2, f))
    src = np.asarray(source, f)
    maps = []
    for c in range(NCORES):
        maps.append(
            {
                "srcT_blk": np.ascontiguousarray(
                    src[c * ROWS : (c + 1) * ROWS].T
                ).astype(bf),
                "tgtT": tgtT,
                "w1a": w1a,
                "w1b": w1b,
                "b1": b1v,
                "w2": w2v,
                "b2": b2v,
            }
        )
    return maps


def _patch_ldw_opt():
    """Rewrite --enable-ldw-opt=false -> true in the walrus invocation
    (enables fast weight load; gated on KERNEL_LDW_OPT=1 for A/B)."""
    import os

    if os.environ.get("KERNEL_LDW_OPT") != "1" or _CACHE.get("ldw_patched"):
        return
    from concourse import bass_utils

    orig = bass_utils.run_command

    def patched(argv, **kwargs):
        argv = [
            "--enable-ldw-opt=true" if a == "--enable-ldw-opt=false" else a
            for a in argv
        ]
        return orig(argv, **kwargs)

    bass_utils.run_command = patched
    _CACHE["ldw_patched"] = True


def run(inputs, trace=False):
    """Run the SPMD kernel; returns ((plan, C), exec_time_ns_or_None)."""
    from concourse import bass_utils

    _patch_ldw_opt()

    nc = _get_nc()
    in_maps = make_in_maps(**inputs)
    res = bass_utils.run_bass_kernel_spmd(
        nc, in_maps, list(range(NCORES)), trace=trace
    )
    _CACHE["last_res"] = res
    plan = np.concatenate([res.results[c]["plan_blk"] for c in range(NCORES)], axis=0)
    C = np.concatenate([res.results[c]["C_blk"] for c in range(NCORES)], axis=0)
    b2v = float(np.asarray(inputs["b2"], np.float64)[0])
    if b2v != 0.0:
        C = C + np.float32(b2v)
    return (plan, C), res.exec_time_ns


def kernel(source, target, W1, b1, W2, b2):
    (plan, C), _ = run(
        dict(source=source, target=target, W1=W1, b1=b1, W2=W2, b2=b2)
    )
    return plan, C


# revision 39
# speedup vs baseline: 4.9576x; 1.2431x over previous
"""Neural optimal transport kernel for 8 TRN2 NeuronCores.

Math (equivalent to the reference):
  hs = source @ W1[:256] + b1 ; ht = target @ W1[256:]
  C[i,j]  = relu(hs[i] + ht[j]) . W2 + b2
  K       = exp(-C)
  u       = (1/n) / (K @ b + eps)          b = ones/n  (constant every iter!)
  v       = (1/n) / (K^T @ u + eps)
  plan    = u[:,None] * K * v[None,:]
The reference's 100-iteration Sinkhorn loop is degenerate: the loop body
uses the constant marginal b (not v), so u and v are identical every
iteration; one evaluation gives the exact fixed point.

Sharding: rows of C/K (the i axis) are split 128-per-core across 8 cores.
K^T@u partial sums are combined with a single 4KB AllReduce.
"""

import numpy as np

N = 1024
EMBED = 256
HID = 128
EPS = 1e-8
NCORES = 8
ROWS = N // NCORES  # 128 rows of C per core
WAVES = ROWS // 4  # 4 rows per psum wave (partitions 0/32/64/96)

# Producer engine pattern, indexed by i % len: S=ScalarE, G=GpSimd, V=VectorE
# (GpSimd measured ~28us per [128,1024] tensor_scalar on HW — never use it)
PRODUCERS = "SVVVSVVVSV"
DEBUG_TAPS = False
WARMUP_CC = True

_CACHE = {}


def _build():
    from contextlib import ExitStack

    import concourse.bass as bass
    import concourse.tile as tile
    from concourse import bacc, mybir

    f32 = mybir.dt.float32
    bf16 = mybir.dt.bfloat16

    nc = bacc.Bacc("TRN2", num_devices=NCORES, debug=False)

    srcT = nc.dram_tensor("srcT_blk", [EMBED, ROWS], bf16, kind="ExternalInput").ap()
    tgtT = nc.dram_tensor("tgtT", [EMBED, N], bf16, kind="ExternalInput").ap()
    w1a = nc.dram_tensor("w1a", [EMBED, HID], bf16, kind="ExternalInput").ap()
    w1b = nc.dram_tensor("w1b", [EMBED, HID], bf16, kind="ExternalInput").ap()
    b1d = nc.dram_tensor("b1", [HID], f32, kind="ExternalInput").ap()
    w2d = nc.dram_tensor("w2", [HID], f32, kind="ExternalInput").ap()
    b2d = nc.dram_tensor("b2", [1], f32, kind="ExternalInput").ap()
    C_out = nc.dram_tensor("C_blk", [ROWS, N], f32, kind="ExternalOutput").ap()
    P_out = nc.dram_tensor("plan_blk", [ROWS, N], f32, kind="ExternalOutput").ap()
    dbg_out = None
    if DEBUG_TAPS:
        dbg_out = nc.dram_tensor("dbg", [3, 128, 8], f32, kind="ExternalOutput").ap()

    from concourse.masks import make_identity

    with tile.TileContext(nc) as tc, ExitStack() as ctx:
        singles = ctx.enter_context(tc.tile_pool(name="singles", bufs=1))
        mpool = ctx.enter_context(tc.tile_pool(name="mpool", bufs=6))
        ps1 = ctx.enter_context(tc.tile_pool(name="ps1", bufs=1, space="PSUM"))
        psct = ctx.enter_context(tc.tile_pool(name="psct", bufs=1, space="PSUM"))
        pstp = ctx.enter_context(tc.tile_pool(name="pstp", bufs=1, space="PSUM"))
        psv = ctx.enter_context(tc.tile_pool(name="psv", bufs=1, space="PSUM"))
        dram = ctx.enter_context(tc.tile_pool(name="dram", bufs=1, space="DRAM"))

        # warm-up AllGather: no data deps, so it schedules early and warms
        # the collective firmware path before the real gather at the tail
        if WARMUP_CC:
            wu_in = dram.tile([128, 1], f32, tag="wu_in")
            wu_out = dram.tile(
                [NCORES * 128, 1], f32, tag="wu_out", addr_space="Shared"
            )
            wu_sb = singles.tile([128, 1], f32, tag="wu_sb")
            nc.vector.memset(wu_sb, 0.0)
            nc.gpsimd.dma_start(out=wu_in[:], in_=wu_sb)
            nc.gpsimd.collective_compute(
                "AllGather",
                mybir.AluOpType.bypass,
                replica_groups=[list(range(NCORES))],
                ins=[wu_in[:].opt()],
                outs=[wu_out[:].opt()],
            )

        # ---- load inputs ------------------------------------------------
        # tgtT is the big input (512KB): split into 4 DMAs so transfers
        # spread across queues and stage-1 matmuls start sooner
        # inputs ride both HWDGE rings (SP=nc.sync, ACT=nc.scalar)
        tgt_sb = singles.tile([128, 2, N], bf16, tag="tgt_sb")
        tgtT_r = tgtT.rearrange("(a p) n -> p a n", p=128)
        for k in range(2):
            for h in range(2):
                sl = slice(512 * h, 512 * h + 512)
                eng = nc.sync if h == 0 else nc.scalar
                eng.dma_start(out=tgt_sb[:, k, sl], in_=tgtT_r[:, k, sl])
        src_sb = singles.tile([128, 2, ROWS], bf16, tag="src_sb")
        nc.scalar.dma_start(out=src_sb, in_=srcT.rearrange("(a p) n -> p a n", p=128))
        w1a_sb = singles.tile([128, 2, HID], bf16, tag="w1a_sb")
        nc.scalar.dma_start(out=w1a_sb, in_=w1a.rearrange("(a p) n -> p a n", p=128))
        w1b_sb = singles.tile([128, 2, HID], bf16, tag="w1b_sb")
        nc.sync.dma_start(out=w1b_sb, in_=w1b.rearrange("(a p) n -> p a n", p=128))
        b1_sb = singles.tile([128, 1], f32, tag="b1_sb")
        nc.sync.dma_start(out=b1_sb, in_=b1d.rearrange("(p a) -> p a", a=1))
        w2f = singles.tile([128, 1], f32, tag="w2f")
        nc.sync.dma_start(out=w2f, in_=w2d.rearrange("(p a) -> p a", a=1))
        b2_sb = singles.tile([128, 1], f32, tag="b2_sb")
        nc.sync.dma_start(out=b2_sb, in_=b2d.partition_broadcast(128))

        w2b = singles.tile([128, 1], bf16, tag="w2b")
        nc.vector.tensor_copy(w2b, w2f)
        negb2 = singles.tile([128, 1], f32, tag="negb2")
        nc.vector.tensor_scalar_mul(negb2, b2_sb, -1.0)

        # ---- stage 1: hsT [h, i_blk], htT [h, j] via PE ------------------
        ht_ps = ps1.tile([128, N], f32, tag="ht_ps")
        for h in range(2):
            sl = slice(512 * h, 512 * h + 512)
            for k in range(2):
                nc.tensor.matmul(
                    out=ht_ps[:, sl],
                    lhsT=w1b_sb[:, k, :],
                    rhs=tgt_sb[:, k, sl],
                    start=(k == 0),
                    stop=(k == 1),
                )
        hs_ps = ps1.tile([128, ROWS], f32, tag="hs_ps")
        for k in range(2):
            nc.tensor.matmul(
                out=hs_ps,
                lhsT=w1a_sb[:, k, :],
                rhs=src_sb[:, k, :],
                start=(k == 0),
                stop=(k == 1),
            )

        htT_b = singles.tile([128, N], bf16, tag="htT_b")
        nc.vector.tensor_copy(htT_b, ht_ps)
        # hs' = hs + b1 (per-partition over h)
        hs_f32 = singles.tile([128, ROWS], f32, tag="hs_f32")
        nc.vector.tensor_scalar_add(hs_f32, hs_ps, b1_sb)

        # ---- stage 2: C^T columns via per-(i, jc) stationary matmuls -----
        # stationary = relu-tile chunk [128_h, 128_j], moving = W2 [128_h, 1]
        # -> out [128_j, 1] = column i of C^T chunk jc, dense in partitions.
        add = mybir.AluOpType.add
        mx = mybir.AluOpType.max
        mult = mybir.AluOpType.mult
        relu = mybir.ActivationFunctionType.Relu

        ident = singles.tile([128, 128], f32, tag="ident")
        make_identity(nc, ident)

        CT_ps = psct.tile([128, N], f32, tag="CT_ps")  # [j%128, jc*128 + i]
        for i in range(ROWS):
            m_t = mpool.tile([128, N], bf16, tag="m_t")
            eng = PRODUCERS[i % len(PRODUCERS)]
            if eng == "S":
                nc.scalar.activation(
                    m_t, htT_b, relu, bias=hs_f32[:, i : i + 1], scale=1.0
                )
            elif eng == "G":
                nc.gpsimd.tensor_scalar(
                    m_t, htT_b, hs_f32[:, i : i + 1], 0.0, add, mx
                )
            else:
                nc.vector.tensor_scalar(
                    m_t, htT_b, hs_f32[:, i : i + 1], 0.0, add, mx
                )
            for jc in range(8):
                nc.tensor.matmul(
                    out=CT_ps[:, 128 * jc + i : 128 * jc + i + 1],
                    lhsT=m_t[:, 128 * jc : 128 * jc + 128],
                    rhs=w2b,
                    start=True,
                    stop=True,
                )

        # evacuate C^T (in halves so transposes start early), transpose via PE
        CT_sb = singles.tile([128, N], f32, tag="CT_sb")
        for h in range(2):
            sl = slice(512 * h, 512 * h + 512)
            nc.vector.tensor_copy(CT_sb[:, sl], CT_ps[:, sl])
        TP_ps = pstp.tile([128, N], f32, tag="TP_ps")
        for jc in range(8):
            nc.tensor.transpose(
                TP_ps[:, 128 * jc : 128 * jc + 128],
                CT_sb[:, 128 * jc : 128 * jc + 128],
                ident,
            )

        # ---- stage 3: K = exp(-(C+b2)) in bf16, fused with row-sum -------
        # (reads the transposed psum directly; C copy for DMA runs parallel)
        K_b = singles.tile([128, N], bf16, tag="K_b")
        rs = singles.tile([128, 1], f32, tag="rs")
        nc.scalar.activation(
            K_b,
            TP_ps,
            mybir.ActivationFunctionType.Exp,
            bias=negb2,
            scale=-1.0,
            accum_out=rs,
        )
        Kpre = singles.tile([128, N], f32, tag="Kpre")  # C - b2, row-major
        nc.vector.tensor_copy(Kpre, TP_ps)
        # C output (without b2; b2 is re-added host-side iff nonzero)
        for h in range(2):
            sl = slice(512 * h, 512 * h + 512)
            eng = nc.sync if h == 0 else nc.scalar
            eng.dma_start(out=C_out[:, sl], in_=Kpre[:, sl])

        # u = (1/n) / (rowsum/n + eps)
        t1 = singles.tile([128, 1], f32, tag="t1")
        nc.vector.tensor_scalar(t1, rs, 1.0 / N, EPS, mult, add)
        rcp1 = singles.tile([128, 1], f32, tag="rcp1")
        nc.vector.reciprocal(rcp1, t1)
        u_sb = singles.tile([128, 1], f32, tag="u_sb")
        nc.vector.tensor_scalar_mul(u_sb, rcp1, 1.0 / N)

        # ---- stage 4: v_denom partial = K^T @ u, AllGather + local sum ---
        cc_in = dram.tile([128, 8], f32, tag="cc_in")
        ag_out = dram.tile([NCORES * 128, 8], f32, tag="ag_out", addr_space="Shared")
        # bf16 matvec: the f32 M=1 matmul path yields wrong results on HW
        u_b = singles.tile([128, 1], bf16, tag="u_b")
        nc.vector.tensor_copy(u_b, u_sb)
        v_row = singles.tile([1, N], f32, tag="v_row")
        for h in range(2):
            vp = psv.tile([1, 512], f32, tag="vp")
            nc.tensor.matmul(
                out=vp, lhsT=u_b, rhs=K_b[:, 512 * h : 512 * h + 512],
                start=True, stop=True,
            )
            nc.vector.tensor_copy(v_row[:, 512 * h : 512 * h + 512], vp)
        # v_row [1, 1024] SBUF -> cc_in [128, 8] DRAM: DMA copies in linear AP
        # order, so the reshape happens via the DRAM-side view. (Folding SBUF
        # free offsets into partitions via rearrange reads garbage on HW.)
        nc.gpsimd.dma_start(out=cc_in[:], in_=v_row)
        nc.gpsimd.collective_compute(
            "AllGather",
            mybir.AluOpType.bypass,
            replica_groups=[list(range(NCORES))],
            ins=[cc_in[:].opt()],
            outs=[ag_out[:].opt()],
        )
        # local sum of the 8 gathered partials; one contiguous 4KB DMA per
        # core block into [128, core, 8], reduce over core (strided inner)
        vg = singles.tile([128, NCORES, 8], f32, tag="vg")
        for a in range(NCORES):
            eng = nc.sync if a % 2 == 0 else nc.scalar
            eng.dma_start(
                out=vg[:, a, :], in_=ag_out[128 * a : 128 * (a + 1), :]
            )
        vd = singles.tile([128, 8], f32, tag="vd")
        nc.vector.tensor_reduce(
            vd,
            vg[:].transpose([0, 2, 1]),
            axis=mybir.AxisListType.X,
            op=mybir.AluOpType.add,
        )
        # v = (1/n) / (v_denom + eps)   computed at [128, 8]
        t2 = singles.tile([128, 8], f32, tag="t2")
        nc.vector.tensor_scalar_add(t2, vd, EPS)
        rcp2 = singles.tile([128, 8], f32, tag="rcp2")
        nc.vector.reciprocal(rcp2, t2)
        v_sb = singles.tile([128, 8], f32, tag="v_sb")
        nc.vector.tensor_scalar_mul(v_sb, rcp2, 1.0 / N)
        # roundtrip through DRAM to reshape [128, 8] -> one [1, 1024] row,
        # then PE-broadcast (ones outer product) to all 128 partitions.
        # both DMAs on the same sync queue -> FIFO ordered
        v_dram = dram.tile([128, 8], f32, tag="v_dram")
        nc.sync.dma_start(out=v_dram[:], in_=v_sb)
        v_row2 = singles.tile([1, N], f32, tag="v_row2")
        nc.sync.dma_start(out=v_row2, in_=v_dram[:].rearrange("p c -> (p c)"))
        v_row_b = singles.tile([1, N], bf16, tag="v_row_b")
        nc.vector.tensor_copy(v_row_b, v_row2)
        ones_b = singles.tile([1, 128], bf16, tag="ones_b")
        nc.vector.memset(ones_b, 1.0)
        # reuses the C^T psum slot (same tag): CT_ps is long dead by now
        V_b = psct.tile([128, N], f32, tag="CT_ps", name="V_b")
        for h in range(2):
            sl = slice(512 * h, 512 * h + 512)
            nc.tensor.matmul(
                out=V_b[:, sl], lhsT=ones_b, rhs=v_row_b[:, sl],
                start=True, stop=True,
            )

        if DEBUG_TAPS:
            # debug taps: local v partial, summed denom, u (repeated x8)
            nc.sync.dma_start(out=dbg_out[0], in_=v_row)
            nc.sync.dma_start(out=dbg_out[1], in_=vd)
            u_rep = singles.tile([128, 8], f32, tag="u_rep")
            nc.vector.tensor_scalar(u_rep, vd, 0.0, u_sb, mult, add)
            nc.sync.dma_start(out=dbg_out[2], in_=u_rep)

        # ---- stage 5: plan = (u * K) * v ---------------------------------
        # u*K has no dependency on the collective result, so the scheduler
        # can run it inside the AllGather window.
        plan1 = singles.tile([128, N], f32, tag="plan1")
        nc.vector.tensor_scalar_mul(plan1, K_b, u_sb)
        plan2 = singles.tile([128, N], f32, tag="plan2")
        nc.vector.tensor_mul(plan2, plan1, V_b)
        for h in range(2):
            sl = slice(512 * h, 512 * h + 512)
            eng = nc.sync if h == 0 else nc.scalar
            eng.dma_start(out=P_out[:, sl], in_=plan2[:, sl])

    nc.compile()
    return nc


def _get_nc():
    if "nc" not in _CACHE:
        _CACHE["nc"] = _build()
    return _CACHE["nc"]


def make_in_maps(source, target, W1, b1, W2, b2):
    import ml_dtypes

    f = np.float32
    bf = ml_dtypes.bfloat16
    tgtT = np.ascontiguousarray(np.asarray(target, f).T).astype(bf)
    w1a = np.ascontiguousarray(np.asarray(W1, f)[:EMBED]).astype(bf)
    w1b = np.ascontiguousarray(np.asarray(W1, f)[EMBED:]).astype(bf)
    b1v = np.ascontiguousarray(np.asarray(b1, f))
    w2v = np.ascontiguousarray(np.asarray(W2, f)[:, 0])
    b2v = np.ascontiguousarray(np.asarray(b2, f))
    src = np.asarray(source, f)
    maps = []
    for c in range(NCORES):
        maps.append(
            {
                "srcT_blk": np.ascontiguousarray(
                    src[c * ROWS : (c + 1) * ROWS].T
                ).astype(bf),
                "tgtT": tgtT,
                "w1a": w1a,
                "w1b": w1b,
                "b1": b1v,
                "w2": w2v,
                "b2": b2v,
            }
        )
    return maps


def _patch_ldw_opt():
    """Rewrite --enable-ldw-opt=false -> true in the walrus invocation
    (enables fast weight load; gated on KERNEL_LDW_OPT=1 for A/B)."""
    import os

    if os.environ.get("KERNEL_LDW_OPT") != "1" or _CACHE.get("ldw_patched"):
        return
    from concourse import bass_utils

    orig = bass_utils.run_command

    def patched(argv, **kwargs):
        argv = [
            "--enable-ldw-opt=true" if a == "--enable-ldw-opt=false" else a
            for a in argv
        ]
        return orig(argv, **kwargs)

    bass_utils.run_command = patched
    _CACHE["ldw_patched"] = True


def run(inputs, trace=False):
    """Run the SPMD kernel; returns ((plan, C), exec_time_ns_or_None)."""
    from concourse import bass_utils

    _patch_ldw_opt()

    nc = _get_nc()
    in_maps = make_in_maps(**inputs)
    res = bass_utils.run_bass_kernel_spmd(
        nc, in_maps, list(range(NCORES)), trace=trace
    )
    _CACHE["last_res"] = res
    plan = np.concatenate([res.results[c]["plan_blk"] for c in range(NCORES)], axis=0)
    C = np.concatenate([res.results[c]["C_blk"] for c in range(NCORES)], axis=0)
    b2v = float(np.asarray(inputs["b2"], np.float64)[0])
    if b2v != 0.0:
        C = C + np.float32(b2v)
    return (plan, C), res.exec_time_ns


def kernel(source, target, W1, b1, W2, b2):
    (plan, C), _ = run(
        dict(source=source, target=target, W1=W1, b1=b1, W2=W2, b2=b2)
    )
    return plan, C
